# revision 1
# baseline (speedup 1.0000x reference)
"""Trainium2 Bass kernel for nn_CrossAttention (B=8, N=4096, C=768, NH=8, 2 views).

Strategy: pure data-parallel over batch B across the 8 NeuronCores (one batch
element per core). Everything on-device runs in "transposed space" (channel dim
on SBUF partitions, tokens on the free axis), so all four projections
(Q, K, V via Wk, and the output projection) are plain K=128-contraction
matmuls with the weights as the stationary operand. The host pre-transposes
the per-core activations ([N, C] -> [C, N]) and weights, which is a pure
layout choice (byte count unchanged) and lets every DMA run at full
contiguity.

Per-token attention over the 2 views reduces to a sigmoid:
  a0 = sigmoid(scale * (q.k0 - q.k1)), a1 = 1 - a0 = sigmoid(-scale * (...)).
The per-head segmented reductions (q.k over each head's 96 dims) and the
per-head broadcast of the attention weights back to 96-wide channel segments
are done on the TensorEngine with tiny constant masks (+-1 selector
matrices), so the VectorEngine only does streaming elementwise work.
"""

from contextlib import ExitStack

import numpy as np
import ml_dtypes

import concourse.bass as bass
import concourse.mybir as mybir
import concourse.tile as tile
from concourse import bacc
from concourse.bass_utils import run_bass_kernel_spmd

B, N, C, NH, HD = 8, 4096, 768, 8, 96
P = 128
KO = C // P            # 6 channel chunks of 128
BLK = 512              # tokens per block
NBLK = N // BLK        # 8 blocks per core
NCORES = 8
SCALE = float(HD) ** -0.5
F32 = mybir.dt.float32
BF16 = mybir.dt.bfloat16

_STATE = {}


def _build_core_kernel(ctx, tc, aps, reps=1):
    nc = tc.nc
    qT, k0T, k1T, v0T, v1T = aps["qT"], aps["k0T"], aps["k1T"], aps["v0T"], aps["v1T"]
    outT = aps["outT"]

    consts = ctx.enter_context(tc.tile_pool(name="consts", bufs=1))
    inp = ctx.enter_context(tc.tile_pool(name="inp", bufs=2))
    att = ctx.enter_context(tc.tile_pool(name="att", bufs=2))
    psum = ctx.enter_context(tc.tile_pool(name="psum", bufs=3, space="PSUM"))
    psl_pool = ctx.enter_context(tc.tile_pool(name="psl", bufs=2, space="PSUM"))
    psb_pool = ctx.enter_context(tc.tile_pool(name="psb", bufs=2, space="PSUM"))

    # Weights, cast to bf16 during the (SWDGE) DMA. Layout [P, KO(c_in), C(c_out)].
    # Emit wqT/wkT first: the first block's Q/K matmuls only wait on those; the
    # remaining constants load behind block 0's activation DMAs.
    w_sb = {}
    for wname in ("wqT", "wkT", "wpT"):
        w_sb[wname] = consts.tile([P, KO, C], BF16, tag=wname, name=wname)

    def _load_w(wname):
        nc.gpsimd.dma_start(
            out=w_sb[wname][:],
            in_=aps[wname].rearrange("(ko p) o -> p ko o", p=P),
        )

    _load_w("wqT")
    _load_w("wkT")
    bias_sb = consts.tile([P, KO], F32, tag="bias")
    hm_sb = consts.tile([P, KO, NH], BF16, tag="hm")
    sel_sb = consts.tile([NH, KO, P], BF16, tag="sel")
    nc.sync.dma_start(hm_sb[:], aps["hm"])

    def _load_late_consts():
        # consumed only by phase_b1/b2, which are emitted after this point
        _load_w("wpT")
        nc.sync.dma_start(bias_sb[:], aps["bias"])
        nc.sync.dma_start(sel_sb[:], aps["sel"])

    qT_r = qT.rearrange("(ko p) n -> p ko n", p=P)
    kT_r = [k0T.rearrange("(ko p) n -> p ko n", p=P),
            k1T.rearrange("(ko p) n -> p ko n", p=P)]
    vT_r = [v0T.rearrange("(ko p) n -> p ko n", p=P),
            v1T.rearrange("(ko p) n -> p ko n", p=P)]
    outT_r = outT.rearrange("(ko p) n -> p ko n", p=P)

    def phase_a(blk):
        """Loads, Q/K/V projections, q*k products, logits accumulation."""
        q_in = inp.tile([P, KO, BLK], BF16, tag="q", name="q")
        nc.gpsimd.dma_start(out=q_in[:], in_=qT_r[:, :, bass.ts(blk, BLK)])
        k_in = []
        v_in = []
        for s in range(2):
            kt = inp.tile([P, KO, BLK], BF16, tag=f"k{s}", name=f"k{s}")
            nc.gpsimd.dma_start(out=kt[:], in_=kT_r[s][:, :, bass.ts(blk, BLK)])
            k_in.append(kt)
            vt = inp.tile([P, KO, BLK], BF16, tag=f"v{s}", name=f"v{s}")
            nc.gpsimd.dma_start(out=vt[:], in_=vT_r[s][:, :, bass.ts(blk, BLK)])
            v_in.append(vt)

        # Q projection: qhat[oc] = sum_ko WqT[ko,oc].T @ q_in[ko]
        qhat = att.tile([P, KO, BLK], BF16, tag="qhat", name="qhat")
        for oc in range(KO):
            ps = psum.tile([P, BLK], F32, tag="proj", name="ps_q")
            for ko in range(KO):
                nc.tensor.matmul(
                    ps[:], w_sb["wqT"][:, ko, bass.ts(oc, P)], q_in[:, ko, :],
                    start=(ko == 0), stop=(ko == KO - 1),
                )
            nc.scalar.copy(qhat[:, oc, :], ps[:])

        # K projection (both views) + q*(k1-k0) product.
        # qkd = qhat .* (khat1 - khat0); per-head sums of qkd give l1 - l0,
        # so a0 = sigmoid(-scale * diff) needs only ONE mask-MM set.
        qkd = att.tile([P, KO, BLK], BF16, tag="qkd", name="qkd")
        kh0 = att.tile([P, KO, BLK], BF16, tag="kh0", name="kh0")
        for oc in range(KO):
            ps = psum.tile([P, BLK], F32, tag="proj", name="ps_k0")
            for ko in range(KO):
                nc.tensor.matmul(
                    ps[:], w_sb["wkT"][:, ko, bass.ts(oc, P)], k_in[0][:, ko, :],
                    start=(ko == 0), stop=(ko == KO - 1),
                )
            nc.vector.tensor_copy(kh0[:, oc, :], ps[:])
        for oc in range(KO):
            ps = psum.tile([P, BLK], F32, tag="proj", name="ps_k1")
            for ko in range(KO):
                nc.tensor.matmul(
                    ps[:], w_sb["wkT"][:, ko, bass.ts(oc, P)], k_in[1][:, ko, :],
                    start=(ko == 0), stop=(ko == KO - 1),
                )
            kd = att.tile([P, BLK], BF16, tag="kd", name="kd")
            nc.vector.tensor_sub(kd[:], ps[:], kh0[:, oc, :])
            nc.vector.tensor_mul(qkd[:, oc, :], kd[:], qhat[:, oc, :])

        # V projection (both views through Wk per the reference).
        # Keep vh1; wd = vhat0 - vhat1 so x = vh1 + a0 .* wd.
        vh1 = att.tile([P, KO, BLK], BF16, tag="vh1", name="vh1")
        wd = att.tile([P, KO, BLK], BF16, tag="wd", name="wd")
        for oc in range(KO):
            ps = psum.tile([P, BLK], F32, tag="proj", name="ps_v1")
            for ko in range(KO):
                nc.tensor.matmul(
                    ps[:], w_sb["wkT"][:, ko, bass.ts(oc, P)], v_in[1][:, ko, :],
                    start=(ko == 0), stop=(ko == KO - 1),
                )
            nc.vector.tensor_copy(vh1[:, oc, :], ps[:])
        for oc in range(KO):
            ps = psum.tile([P, BLK], F32, tag="proj", name="ps_v0")
            for ko in range(KO):
                nc.tensor.matmul(
                    ps[:], w_sb["wkT"][:, ko, bass.ts(oc, P)], v_in[0][:, ko, :],
                    start=(ko == 0), stop=(ko == KO - 1),
                )
            nc.vector.tensor_sub(wd[:, oc, :], ps[:], vh1[:, oc, :])

        # logits diff: psl[h, n] = sum_c qkd[c, n] over head h  (= l1 - l0)
        psl = psl_pool.tile([NH, BLK], F32, tag="logits", name="psl")
        for oc in range(KO):
            nc.tensor.matmul(
                psl[:], hm_sb[:, oc, :], qkd[:, oc, :],
                start=(oc == 0), stop=(oc == KO - 1),
            )
        return blk, psl, vh1, wd

    def phase_b1(state):
        """Sigmoid, per-head broadcast (PE), weighted combine (DVE)."""
        blk, psl, vh1, wd = state
        # psl holds l1 - l0 (scale-less); a0 = sigmoid(scale * (l0 - l1))
        a = att.tile([NH, BLK], BF16, tag="a", name="a")
        nc.scalar.activation(a[:], psl[:],
                             mybir.ActivationFunctionType.Sigmoid, scale=-SCALE)

        x = att.tile([P, KO, BLK], BF16, tag="x", name="x")
        for oc in range(KO):
            b_ps = psb_pool.tile([P, BLK], F32, tag="bc0", name="bc0")
            nc.tensor.matmul(b_ps[:], sel_sb[:, oc, :], a[:],
                             start=True, stop=True)
            t0 = att.tile([P, BLK], BF16, tag="t0", name="t0")
            nc.vector.tensor_mul(t0[:], b_ps[:], wd[:, oc, :])
            nc.vector.tensor_add(x[:, oc, :], t0[:], vh1[:, oc, :])
        return blk, x

    def phase_b2(state):
        """Output projection + bias + store."""
        blk, x = state
        out_sb = att.tile([P, KO, BLK], F32, tag="out", name="out_sb")
        for oc in range(KO):
            ps = psum.tile([P, BLK], F32, tag="proj", name="ps_o")
            for ko in range(KO):
                nc.tensor.matmul(
                    ps[:], w_sb["wpT"][:, ko, bass.ts(oc, P)], x[:, ko, :],
                    start=(ko == 0), stop=(ko == KO - 1),
                )
            nc.vector.tensor_scalar_add(out_sb[:, oc, :], ps[:],
                                        bias_sb[:, bass.ts(oc, 1)])
        nc.sync.dma_start(out=outT_r[:, :, bass.ts(blk, BLK)], in_=out_sb[:])

    # 3-stage software pipeline. Per-block PE order becomes
    #   ... A(b+1) | bc(b) | A(b+2) | P-proj(b) ...
    # so the DVE combine of block b runs concurrently with A(b+2)'s matmuls
    # and the in-order PE never waits on sigmoid/combine latency.
    st_a = [phase_a(0)]
    _load_late_consts()
    st_a.append(phase_a(1))
    st_b = [phase_b1(st_a[0])]
    blocks = [(rep, blk) for rep in range(reps) for blk in range(NBLK)]
    for _, blk in blocks[2:]:
        st_a.append(phase_a(blk))
        phase_b2(st_b[-1])
        st_b.append(phase_b1(st_a[-2]))
    phase_b2(st_b[-1])
    st_b.append(phase_b1(st_a[-1]))
    phase_b2(st_b[-1])


def build_program(reps=1):
    nc = bacc.Bacc("TRN2", debug=False, target_bir_lowering=False)
    aps = {}
    for name in ("qT", "k0T", "k1T", "v0T", "v1T"):
        aps[name] = nc.dram_tensor(name, [C, N], F32, kind="ExternalInput").ap()
    for name in ("wqT", "wkT", "wpT"):
        aps[name] = nc.dram_tensor(name, [C, C], F32, kind="ExternalInput").ap()
    aps["bias"] = nc.dram_tensor("bias", [P, KO], F32, kind="ExternalInput").ap()
    aps["hm"] = nc.dram_tensor("hm", [P, KO, NH], BF16, kind="ExternalInput").ap()
    aps["sel"] = nc.dram_tensor("sel", [NH, KO, P], BF16, kind="ExternalInput").ap()
    aps["outT"] = nc.dram_tensor("outT", [C, N], F32, kind="ExternalOutput").ap()

    with tile.TileContext(nc) as tc, ExitStack() as ctx:
        _build_core_kernel(ctx, tc, aps, reps=reps)
    nc.compile()
    return nc


def _get_program():
    if "nc" not in _STATE:
        _STATE["nc"] = build_program()
    return _STATE["nc"]


def make_host_constants(bp):
    bf = ml_dtypes.bfloat16
    heads = np.arange(C) // HD                      # [C]
    bias = np.ascontiguousarray(
        np.asarray(bp, np.float32).reshape(KO, P).T)  # [P, KO]
    hm = np.zeros((C, NH), np.float32)
    for h in range(NH):
        hm[heads == h, h] = 1.0
    hm = np.ascontiguousarray(
        hm.reshape(KO, P, NH).transpose(1, 0, 2)).astype(bf)  # [P, KO, NH]
    sel = np.zeros((NH, C), np.float32)
    for h in range(NH):
        sel[h, heads == h] = 1.0
    sel = np.ascontiguousarray(
        sel.reshape(NH, KO, P)).astype(bf)           # [NH, KO, P]
    return bias, hm, sel


def make_in_maps(query, key, value, Wq, Wk, Wp, bp):
    query = np.asarray(query, np.float32)
    key = np.asarray(key, np.float32)
    value = np.asarray(value, np.float32)
    wqT = np.ascontiguousarray(np.asarray(Wq, np.float32).T)
    wkT = np.ascontiguousarray(np.asarray(Wk, np.float32).T)
    wpT = np.ascontiguousarray(np.asarray(Wp, np.float32).T)
    bias, hm, sel = make_host_constants(bp)
    in_maps = []
    for b in range(NCORES):
        in_maps.append({
            "qT": np.ascontiguousarray(query[b].T),
            "k0T": np.ascontiguousarray(key[b, :, 0, :].T),
            "k1T": np.ascontiguousarray(key[b, :, 1, :].T),
            "v0T": np.ascontiguousarray(value[b, :, 0, :].T),
            "v1T": np.ascontiguousarray(value[b, :, 1, :].T),
            "wqT": wqT, "wkT": wkT, "wpT": wpT,
            "bias": bias, "hm": hm, "sel": sel,
        })
    return in_maps


def run(query, key, value, Wq, Wk, Wp, bp, trace=False, **trace_kwargs):
    nc = _get_program()
    in_maps = make_in_maps(query, key, value, Wq, Wk, Wp, bp)
    res = run_bass_kernel_spmd(nc, in_maps, list(range(NCORES)),
                               trace=trace, **trace_kwargs)
    out = np.stack([np.ascontiguousarray(r["outT"]).T for r in res.results])
    return out.astype(np.float32), res


def kernel(query, key, value, Wq, Wk, Wp, bp):
    out, _ = run(query, key, value, Wq, Wk, Wp, bp)
    return out

